# revision 26
# baseline (speedup 1.0000x reference)
"""AR(24) extrapolation kernel for Trainium2 (8 NeuronCores, data parallel).

The reference runs a 168-step scalar-weight autoregressive recurrence over the
last 24 timesteps of x, independently per (batch, channel).  Because the
recurrence is linear, output step t is a fixed linear combination of the
initial 24-sample window plus a bias term:

    y[b, t, d] = sum_i C[i, t] * x[b, S-24+i, d] + beta[t]

C [24, 168] and beta [168] follow from W/b by unrolling the recurrence once on
the host (float64, ~4k flops).  The device work is then a memory-bound
broadcast matmul: per core, out[t, (b, d)] = CB^T @ xaug where CB stacks
[C; beta] and xaug stacks [x_window^T; ones].

Sharding: pure data parallelism over batch (32 batches per core).

Measured facts this version is built around (trn2, this part):
- The PE runs its four 32-row strips CONCURRENTLY at ~1.2 GHz each (clock
  stuck at mid p-state; ~4.6 cols/ns aggregate measured) -> the PE is never
  the bottleneck if >=2 matmuls from different strips are in flight.
- PSUM->SBUF copies run at ~1 col/ns (DVE) / ~1.2 col/ns (Act) + ~200 ns/op;
  the 21504 copy columns over two engines (~14 us) sit just above the fp16
  store stream (~5.5 MB / ~400 GB/s ~ 14 us).  Both engines must therefore do
  NOTHING but copies, interleaved strictly pair-by-pair (no head-of-line
  blocking), with DMA triggers moved to the SP engine (HWDGE) and the GpSimd
  engine (SWDGE).
- Outputs are staged and stored as fp16 with a power-of-2 output scale folded
  into CB (PSUM holds y*so); the host multiplies by 1/so while gathering.

Layout details per core:
- input xpack [128, 4096]: 32 moving blocks of [25, 512] (24 window rows plus
  a ones row); block b sits at row strip 32*(b%4), columns (b//4)*512.
- weights cb [25, 168] in DRAM, replicated into the 4 PE row strips via 4
  small 25-row DMAs (strip 0 first - the first matmul gates on it).
- phase A (t 0..127, [t, d] orientation): per batch pair one 2-bank PSUM tile
  [128, 2, 512]; one downconverting copy per pair.
- phase B (t 128..167, transposed [d%128, (d//128, t')] orientation): per
  batch pair one 1-bank PSUM tile [128, 2, 160]; one strided copy per pair.
- copies alternate engines per pair: pair k sends its A-copy to engine k%2
  and its B-copy to the other; each engine sees exactly one copy per pair in
  pair order.
- stores: out [128, NB*D] ([t, b*D+d]) and outt [128, NB*4*40] fp16; each sub
  issues one store per tensor, alternating the SP HWDGE ring and the GpSimd
  SWDGE ring per sub so both queues carry ~half the bytes.
"""

import numpy as np

import concourse.bacc as bacc
import concourse.tile as tile
from concourse import mybir
from concourse.bass_utils import run_bass_kernel_spmd

ORDER = 24
K = ORDER + 1            # contraction: 24 window rows + ones row
T = 168
D = 512
B = 256
S = 336
N_CORES = 8
NB = B // N_CORES        # 32 local batches per core
COLS = NB * D
P0 = 128
P1 = T - P0              # 40
GROUPS = [4, 4, 8, 8, 8]  # input-load groups
SUBS = [2, 2, 4, 8, 8, 4, 2, 2]  # store chunks: small at both ends so the
                                  # stream starts early and drains fast
assert sum(GROUPS) == NB and sum(SUBS) == NB
F32 = mybir.dt.float32
F16 = mybir.dt.float16

_nc_cache = None


def _copy(eng, out, in_):
    if hasattr(eng, "tensor_copy"):
        eng.tensor_copy(out, in_)
    else:
        eng.copy(out, in_)


def _build_program():
    nc = bacc.Bacc()
    xp = nc.declare_dram_parameter("xpack", [128, (NB // 4) * D], F16, isOutput=False)
    cb = nc.declare_dram_parameter("cb", [128, T], F16, isOutput=False)
    # merged output: batch j owns 672 cols = [512 phase-A (t 0..127, [t,d])
    # then 160 phase-B tail (transposed, [d%128, (d//128, t-128)])]
    W0 = D + 4 * P1
    out = nc.declare_dram_parameter("out", [128, NB * W0], F16, isOutput=True)

    with tile.TileContext(nc) as tc:
        with (
            tc.tile_pool(name="consts", bufs=1) as consts,
            tc.tile_pool(name="xin", bufs=1) as xin,
            tc.tile_pool(name="stage", bufs=4) as stage,
            tc.tile_pool(name="psum", bufs=4, space="PSUM") as psum,
        ):
            # ramp-critical loads, one per ring, first in each queue: the
            # first batch pair (strips 0/1) needs cb rows 0:64 and xt0 rows
            # 0:64; the second pair's halves ride right behind them.
            cb_t = consts.tile([128, T], F16)
            xt0 = xin.tile([128, 2 * D], F16, tag="xt0", name="xt0")
            nc.sync.dma_start(out=cb_t[0:64, :], in_=cb[0:64, :])
            nc.scalar.dma_start(out=xt0[0:64, :], in_=xp[0:64, 0 : 2 * D])
            nc.sync.dma_start(out=xt0[64:128, :], in_=xp[64:128, 0 : 2 * D])
            nc.scalar.dma_start(out=cb_t[64:128, :], in_=cb[64:128, :])
            xts = [xt0]
            for g in range(1, 4):
                xt = xin.tile([128, 2 * D], F16, tag=f"xt{g}", name=f"xt{g}")
                src = xp[:, g * 2 * D : (g + 1) * 2 * D]
                eng = nc.sync if g % 2 == 1 else nc.scalar
                eng.dma_start(out=xt, in_=src)
                xts.append(xt)

            def xsrc(j):
                rs = 32 * ((j // 2) % 4)
                cs = (j % 2) * D
                return xts[j // 8][rs : rs + K, cs : cs + D]

            sub0 = 0
            for nsub, sub in enumerate(SUBS):
                st = stage.tile([P0, sub, W0], F16, tag="st", name=f"st_{nsub}")
                for jj0 in range(0, sub, 2):
                    pk = (sub0 + jj0) // 2  # global pair index
                    # phase A: 2-bank pair tile, rotation depth 3 so the PE
                    # refills a free tile while both copy engines drain
                    psA = psum.tile(
                        [P0, 2, D], F32, tag="psA", bufs=3, name=f"psA_{pk}"
                    )
                    rs = 32 * (pk % 4)
                    for k in range(2):
                        j = sub0 + jj0 + k
                        nc.tensor.matmul(
                            psA[:, k, :],
                            cb_t[rs : rs + K, 0:P0],
                            xsrc(j),
                            start=True,
                            stop=True,
                            tile_position=(rs, 0),
                        )
                    # phase B: both batches inside ONE bank-aligned bank
                    # ([128, 2, 256] = exactly 2 KB/partition)
                    psB = psum.tile(
                        [P0, 2, 256], F32, tag="psB", bufs=2, name=f"psB_{pk}"
                    )
                    for q in range(4):
                        for k in range(2):
                            j = sub0 + jj0 + k
                            nc.tensor.matmul(
                                psB[:, k, q * P1 : (q + 1) * P1],
                                xsrc(j)[:, 128 * q : 128 * (q + 1)],
                                cb_t[rs : rs + K, P0:T],
                                start=True,
                                stop=True,
                                tile_position=(rs, 0),
                            )
                    # two copies per pair on opposite engines; each engine
                    # sees exactly one copy per pair, in pair order
                    engA, engB = (
                        (nc.vector, nc.scalar) if pk % 2 == 0 else (nc.scalar, nc.vector)
                    )
                    _copy(engA, st[:, jj0 : jj0 + 2, 0:D], psA[:, :, :])
                    _copy(engB, st[:, jj0 : jj0 + 2, D:W0], psB[:, :, 0 : 4 * P1])

                # ONE merged store per sub, alternating HWDGE rings
                eng = nc.sync if nsub % 2 == 0 else nc.scalar
                eng.dma_start(
                    out=out[:, sub0 * W0 : (sub0 + sub) * W0],
                    in_=st[:, :, :].rearrange("p a b -> p (a b)"),
                )
                sub0 += sub

    nc.finalize()
    return nc


def _unroll_coeffs(W: np.ndarray, b: np.ndarray) -> np.ndarray:
    """Unroll the linear AR recurrence: CB[k, t] with rows 0..23 = window
    coefficients, row 24 = additive bias per step."""
    w = W[:, 0].astype(np.float64)
    bb = float(np.asarray(b).reshape(-1)[0])
    M = np.eye(ORDER)
    m = np.zeros(ORDER)
    CB = np.zeros((K, T), np.float64)
    for t in range(T):
        c = M.T @ w
        yb = m @ w + bb
        CB[:ORDER, t] = c
        CB[ORDER, t] = yb
        M = np.vstack([M[1:], c[None, :]])
        m = np.concatenate([m[1:], [yb]])
    return CB.astype(np.float32)


def _pack_inputs(x: np.ndarray) -> np.ndarray:
    """[N_CORES, 128, (NB//4)*D]: local batch j at row strip 32*((j//2)%4)
    (a PAIR shares one strip so its PSUM tail writes may share a bank; pair
    index rotates strips for PE concurrency), col slot (2*(j//8)+(j%2))*D;
    contents = 24 window rows + a ones row."""
    xw = x[:, -ORDER:, :]
    packed = np.zeros((N_CORES, 128, (NB // 4) * D), np.float32)
    for c in range(N_CORES):
        for j in range(NB):
            rs = 32 * ((j // 2) % 4)
            cs = (2 * (j // 8) + (j % 2)) * D
            packed[c, rs : rs + ORDER, cs : cs + D] = xw[c * NB + j]
            packed[c, rs + ORDER, cs : cs + D] = 1.0
    return packed


def _make_in_maps(x, W, b):
    CB = _unroll_coeffs(W, b)
    packed = _pack_inputs(x)

    maxx = max(float(np.abs(packed).max()), 1.0)
    # output scale so = 2^-m so |y*so| stays well inside fp16 staging range
    ybound = float(np.abs(CB).sum(axis=0).max()) * maxx
    so = 2.0 ** np.floor(np.log2(16384.0 / max(ybound, 1e-30)))
    so = min(so, 1.0)
    CBo = CB * so

    # fp16 operands with a folded power-of-2 balance scale: the device
    # computes (CB*so/s)^T @ (x*s) whose products equal so*CB^T@x exactly,
    # while both operands stay well inside fp16 range (geometric-mean split)
    maxc = max(float(np.abs(CBo).max()), 1e-30)
    s = 2.0 ** round((np.log2(maxc) - np.log2(maxx)) / 2.0)
    assert maxc / s < 3.0e4 and maxx * s < 3.0e4, "fp16 range exceeded"

    CBrep = np.zeros((128, T), np.float16)
    for st_ in range(4):
        CBrep[32 * st_ : 32 * st_ + K] = (CBo / s).astype(np.float16)
    packed16 = (packed * s).astype(np.float16)
    return [{"xpack": packed16[c], "cb": CBrep} for c in range(N_CORES)], so


def kernel(x, W, b, tar_seq_len):
    global _nc_cache
    x = np.asarray(x, dtype=np.float32)
    W = np.asarray(W, dtype=np.float32)
    b = np.asarray(b, dtype=np.float32)
    assert int(tar_seq_len) == T, f"compiled for tar_seq_len={T}"
    assert x.shape == (B, S, D)

    in_maps, so = _make_in_maps(x, W, b)

    if _nc_cache is None:
        _nc_cache = _build_program()
    nc = _nc_cache
    res = run_bass_kernel_spmd(nc, in_maps, list(range(N_CORES)))

    # gather: out [128, NB*672]; per batch j: cols j*672+0:512 = [t, d] for
    # t<128; cols j*672+512:672 = transposed tail [d%128, (d//128, t-128)];
    # undo the output scale exactly
    inv = np.float32(1.0 / so)
    W0 = D + 4 * P1
    parts = []
    for r in res.results:
        o = r["out"].astype(np.float32).reshape(128, NB, W0)
        y = np.empty((NB, T, D), np.float32)
        y[:, 0:P0, :] = o[:, :, 0:D].transpose(1, 0, 2)
        tail = o[:, :, D:W0].reshape(128, NB, 4, P1)
        y[:, P0:T, :] = tail.transpose(1, 3, 2, 0).reshape(NB, P1, D)
        y *= inv
        parts.append(y)
    return np.ascontiguousarray(np.concatenate(parts, axis=0))
